# revision 35
# baseline (speedup 1.0000x reference)
"""Bahdanau-attention scoring kernel for Trainium2 (8 NeuronCores, data-parallel over batch).

Computes, for enc [S=2048, B=64, F=1024] f32 and hidden [B, 512] f32:
    energy    = tanh(cat([hidden_bcast, enc]) @ attn_w.T + attn_b)   # [S, B, 512]
    attention = energy @ v_w.T (+ v_b)                                # [S, B, 1]
    out       = softmax_over_S(attention / TEMP)                      # [S, B, 1]

v_b is a global scalar shift -> cancels in the softmax, dropped.
The hidden part of the concat is S-invariant: h_proj = hidden @ attn_w[:, :512].T + attn_b
is precomputed on host (33 MFLOP) and folded into the tanh bias on device (b fixed per tile).

Per-core layout (b-blocked): for each of 8 local batch rows b, 4 s-tiles of 512 tokens.
enc rows are cast-DMA'd f32->fp16 (gpsimd SWDGE) in natural [token, feature] layout; the
[token, feature] -> [feature, token] transpose required by the TensorE contraction runs as
PE transpose-mode matmuls (KPE=8 k-chunks per tile, packed 8-per-PSUM-bank, copied to SBUF
by DVE/ACT with a cast to the matmul dtype). Energy matmuls run fp8e4m3 DoubleRow (2 MACs/
cell, K=256 per matmul) with weights prescaled x32 (attn_w magnitudes are subnormal in
e4m3); the 1/32 rides the tanh activation's scale. The v-dot runs as M=1 fp16 matmuls
accumulating a [1, 512] attention row per tile; softmax is per-b row sums + reciprocal,
with the final scale split across DVE and ACT. Per-core output is [b, s]-major; the host
transposes (64 KB) and stacks. Measured: ~235 us HW, absmax-rel err ~1.1e-2 (fp8) or
~347 us, ~1.5e-4 with KERNEL_FP8=0 (fp16 everywhere).
"""
import os
import sys
import types

import numpy as np
import ml_dtypes

S = 2048
B = 64
F = 1024
D = 512
NCORES = 8
BLOC = B // NCORES  # 8
TEMP = 3.0
ST = 4          # s-tiles per batch row (S / 512)
TT = 512        # tokens per tile
KCH = F // 128  # 8 contraction chunks
DCH = D // 128  # 4 output-feature chunks
KPE = int(os.environ.get("KERNEL_KPE", "8"))  # k-chunks transposed on PE (rest: DMA xbar)
FP8 = bool(int(os.environ.get("KERNEL_FP8", "1")))  # fp8e4m3 DoubleRow energy matmuls
WSCALE = 32.0  # fp8 weight prescale (attn_w values are subnormal in e4m3 otherwise)


def _install_ntff_hook():
    """Make trace=True work under axon by registering the NTFF profile hook."""
    try:
        from antenv import axon_hooks  # noqa: F401
        return
    except ImportError:
        pass
    try:
        import antenv
        from trn_agent_boot.trn_boot import _ntff_profile_via_ctypes
        mod = types.ModuleType("antenv.axon_hooks")
        mod._hook = _ntff_profile_via_ctypes("/opt/axon/libaxon_pjrt.so")
        mod.set_axon_ntff_profile_hook = lambda h: setattr(mod, "_hook", h)
        mod.get_axon_ntff_profile_hook = lambda: mod._hook
        sys.modules["antenv.axon_hooks"] = mod
        antenv.axon_hooks = mod
    except Exception:
        pass


_NC_CACHE = {}


def _build():
    if "nc" in _NC_CACHE:
        return _NC_CACHE["nc"]
    import concourse.bacc as bacc
    import concourse.mybir as mybir
    from concourse.tile import TileContext
    from concourse.masks import make_identity

    f32 = mybir.dt.float32
    bf16 = mybir.dt.float16  # fp16: same PE/DMA speed as bf16, 8x finer mantissa
    fp8 = mybir.dt.float8e4
    xdt = fp8 if FP8 else bf16

    nc = bacc.Bacc("TRN2")
    enc = nc.dram_tensor("enc", [S, BLOC, F], f32, kind="ExternalInput")
    wt_shape = [128, 2, KCH // 2, D] if FP8 else [128, KCH, D]
    wt = nc.dram_tensor("wt", wt_shape, xdt, kind="ExternalInput")
    hb = nc.dram_tensor("hb", [128, DCH, BLOC], f32, kind="ExternalInput")
    vw = nc.dram_tensor("vw", [128, DCH], bf16, kind="ExternalInput")
    out = nc.dram_tensor("out", [BLOC, S], f32, kind="ExternalOutput")

    # enc viewed so [st, b] indexes a [p=128, q=4, f=1024] tile of 512 tokens
    enc_v = enc.rearrange("(st q p) b f -> st b p q f", q=ST, p=128)

    tiles = [(b, st) for b in range(BLOC) for st in range(ST)]

    with TileContext(nc) as tc:
        with (
            tc.tile_pool(name="consts", bufs=1) as cpool,
            tc.tile_pool(name="work", bufs=1) as pool,
            tc.tile_pool(name="ps_e", bufs=3, space="PSUM") as pse,
            tc.tile_pool(name="ps_a", bufs=2, space="PSUM") as psa,
            tc.tile_pool(name="ps_t", bufs=3, space="PSUM") as pst_pool,
        ):
            wt_sb = cpool.tile(wt_shape, xdt)
            nc.sync.dma_start(out=wt_sb[:], in_=wt[:])
            hb_sb = cpool.tile([128, DCH, BLOC], f32)
            nc.sync.dma_start(out=hb_sb[:], in_=hb[:])
            vw_sb = cpool.tile([128, DCH], bf16)
            nc.sync.dma_start(out=vw_sb[:], in_=vw[:])
            ident = cpool.tile([128, 128], bf16)
            make_identity(nc, ident[:])

            ex_tiles = {}
            part_tiles = {}

            def load_tile(idx):
                b, st = tiles[idx]
                # FP8: load X as fp8 directly; transposes then move fp8 PAIRS as
                # fp16 units (half the PE transpose cycles + half the copy elements),
                # landing each f-pair on one partition = DoubleRow's [Ki, Ko=2] layout.
                xa = pool.tile([128, ST, F], xdt, tag="xa", bufs=4, name="xa")
                for q in range(ST):  # per-q cast DMAs: fine-grained deps keep PE fed
                    nc.gpsimd.dma_start(out=xa[:, q], in_=enc_v[st, b, :, q])
                if FP8:
                    xt = pool.tile([128, ST, KCH // 2, 256], xdt, tag="xt", bufs=2, name="xt")
                else:
                    xt = pool.tile([128, ST, KCH, 128], xdt, tag="xt", bufs=2, name="xt")
                return xa, xt

            def pe_transpose_group(xa, xt, q):
                # PE transpose-mode in 2-byte units; KPC chunks packed per PSUM bank
                kpc = KCH // 2 if FP8 else KCH
                pst = pst_pool.tile([128, kpc, 128], bf16, tag="pst", name="pst")
                src = xa[:, q].bitcast(bf16) if FP8 else xa[:, q]
                for k in range(kpc):
                    nc.tensor.transpose(
                        pst[:, k, :], src[:, k * 128 : (k + 1) * 128], ident[:]
                    )
                dst = xt[:, q].bitcast(bf16) if FP8 else xt[:, q]
                if q == ST - 1:  # rebalance: give ACT a share of the psum->sbuf copies
                    nc.scalar.copy(dst, pst[:])
                else:
                    nc.vector.tensor_copy(dst, pst[:])

            def emit_deferred(idx, ebs):
                b, st = tiles[idx]
                att = psa.tile([1, TT], mybir.dt.float32, tag="att", name="att")
                for dc in range(DCH):
                    nc.tensor.matmul(
                        att[:],
                        lhsT=vw_sb[:, dc : dc + 1],
                        rhs=ebs[dc][:],
                        start=(dc == 0),
                        stop=(dc == DCH - 1),
                    )
                if st == 0:
                    ex_tiles[b] = pool.tile(
                        [1, ST, TT], mybir.dt.float32, tag="ex", bufs=2, name=f"ex{b}"
                    )
                    part_tiles[b] = pool.tile(
                        [1, ST, 2], mybir.dt.float32, tag="parts", bufs=2, name=f"pt{b}"
                    )
                ex_b = ex_tiles[b]
                parts_b = part_tiles[b]
                # halved 1-lane ops: keep ACT/DVE queues fine-grained
                hh = TT // 2
                for hi in range(2):
                    sl = slice(hi * hh, (hi + 1) * hh)
                    nc.scalar.activation(
                        ex_b[:, st, sl], att[:, sl], mybir.ActivationFunctionType.Exp,
                        scale=float(1.0 / TEMP),
                    )
                    nc.vector.reduce_sum(
                        out=parts_b[:, st, hi : hi + 1], in_=ex_b[:, st, sl],
                        axis=mybir.AxisListType.X,
                    )
                if st == ST - 1:
                    sums = pool.tile([1, 1], mybir.dt.float32, tag="sums", bufs=2, name=f"sm{b}")
                    nc.vector.reduce_sum(
                        out=sums[:], in_=parts_b.rearrange("p st h -> p (st h)"),
                        axis=mybir.AxisListType.X,
                    )
                    rec = pool.tile([1, 1], mybir.dt.float32, tag="rec", bufs=2, name=f"rc{b}")
                    nc.vector.reciprocal(rec[:], sums[:])
                    exf = pool.tile([1, ST, TT], mybir.dt.float32, tag="exf", bufs=2, name=f"xf{b}")
                    qlen = (ST * TT) // 4
                    exv = ex_b.rearrange("p st t -> p (st t)")
                    xfv = exf.rearrange("p st t -> p (st t)")
                    # quarter the row scale, alternating DVE/ACT, to shorten the
                    # serial finalize chain (tail-critical for the last b)
                    for qi in range(4):
                        sl = slice(qi * qlen, (qi + 1) * qlen)
                        if qi % 2 == 0:
                            nc.vector.tensor_scalar_mul(xfv[:, sl], exv[:, sl], rec[:])
                        else:
                            nc.scalar.mul(xfv[:, sl], exv[:, sl], rec[:])
                    nc.sync.dma_start(out=out[b : b + 1, :], in_=xfv[:])

            mybir_ref = mybir  # close over for helpers above

            cur = load_tile(0)
            nxt = load_tile(1)
            deferred = None
            for idx in range(len(tiles)):
                b, st = tiles[idx]
                xa, xt = cur
                ebs = []
                for dc in range(DCH):
                    ps = pse.tile([128, TT], mybir_ref.dt.float32, tag="ps", name="ps")
                    if idx == 0 and dc == 0:
                        # startup: interleave tile 0's transposes with per-q matmuls so
                        # the first MM only waits on q0's load+transpose, not the whole tile
                        for q in range(ST):
                            pe_transpose_group(xa, xt, q)
                            if FP8:
                                for kp in range(KCH // 2):
                                    nc.tensor.matmul(
                                        ps[:, q * 128 : (q + 1) * 128],
                                        lhsT=wt_sb[:, :, kp, 0:128],
                                        rhs=xt[:, q, kp, :].rearrange(
                                            "p (t j) -> p j t", j=2
                                        ),
                                        start=(q == 0 and kp == 0),
                                        stop=(q == ST - 1 and kp == KCH // 2 - 1),
                                        perf_mode=mybir_ref.MatmulPerfMode.DoubleRow,
                                    )
                            else:
                                for k in range(KCH):
                                    nc.tensor.matmul(
                                        ps[:, q * 128 : (q + 1) * 128],
                                        lhsT=wt_sb[:, k, 0:128],
                                        rhs=xt[:, q, k, :],
                                        start=(q == 0 and k == 0),
                                        stop=(q == ST - 1 and k == KCH - 1),
                                    )
                    elif FP8:
                        for kp in range(KCH // 2):
                            nc.tensor.matmul(
                                ps[:],
                                lhsT=wt_sb[:, :, kp, dc * 128 : (dc + 1) * 128],
                                rhs=xt[:, :, kp, :].rearrange(
                                    "p q (t j) -> p j q t", j=2
                                ),
                                start=(kp == 0),
                                stop=(kp == KCH // 2 - 1),
                                perf_mode=mybir_ref.MatmulPerfMode.DoubleRow,
                            )
                    else:
                        for k in range(KCH):
                            nc.tensor.matmul(
                                ps[:],
                                lhsT=wt_sb[:, k, dc * 128 : (dc + 1) * 128],
                                rhs=xt[:, :, k, :],
                                start=(k == 0),
                                stop=(k == KCH - 1),
                            )
                    eb = pool.tile([128, TT], mybir_ref.dt.float16, tag="eb", bufs=9, name="eb")
                    nc.scalar.activation(
                        eb[:], ps[:], mybir_ref.ActivationFunctionType.Tanh,
                        bias=hb_sb[:, dc, b : b + 1],
                        scale=(1.0 / WSCALE) if FP8 else 1.0,
                    )
                    ebs.append(eb)
                    # interleave next tile's transposes between MM groups (HAM-friendly)
                    if nxt is not None:
                        pe_transpose_group(nxt[0], nxt[1], dc)
                    if dc == 1 and deferred is not None:
                        emit_deferred(*deferred)
                        deferred = None
                deferred = (idx, ebs)
                cur = nxt
                nxt = load_tile(idx + 2) if idx + 2 < len(tiles) else None
            emit_deferred(*deferred)

    nc.compile()
    _NC_CACHE["nc"] = nc
    return nc


def _prep_consts(hidden, attn_w, attn_b, v_w):
    # h_proj[b, d] = hidden[b] @ attn_w[:, :D].T + attn_b
    h_proj = hidden.astype(np.float64) @ attn_w[:, :D].T.astype(np.float64) + attn_b
    h_proj = h_proj.astype(np.float32)  # [B, D]
    w_e = attn_w[:, D:]  # [D, F]
    if FP8:
        import concourse.mybir as mybir
        # pair-major: wt[p, j, kp, d] = w_e[d, 256*kp + 2*p + j] * WSCALE
        wt = np.ascontiguousarray(
            w_e.T.reshape(KCH // 2, 128, 2, D).transpose(1, 2, 0, 3)
        )
        wt = (wt * WSCALE).astype(mybir.dt.np(mybir.dt.float8e4))
    else:
        # wt[p, k, d] = w_e[d, 128k + p]
        wt = np.ascontiguousarray(w_e.T.reshape(KCH, 128, D).transpose(1, 0, 2))
        wt = wt.astype(np.float16)
    # vw[p, dc] = v_w[0, 128*dc + p]
    vw = np.ascontiguousarray(v_w.reshape(DCH, 128).T).astype(np.float16)
    return h_proj, wt, vw


def kernel(hidden, encoder_outputs, attn_w, attn_b, v_w, v_b):
    _install_ntff_hook()
    from concourse.bass_utils import run_bass_kernel_spmd

    hidden = np.asarray(hidden, dtype=np.float32)
    encoder_outputs = np.asarray(encoder_outputs, dtype=np.float32)
    attn_w = np.asarray(attn_w, dtype=np.float32)
    attn_b = np.asarray(attn_b, dtype=np.float32)
    v_w = np.asarray(v_w, dtype=np.float32)

    nc = _build()
    h_proj, wt, vw = _prep_consts(hidden, attn_w, attn_b, v_w)

    in_maps = []
    for c in range(NCORES):
        b0 = c * BLOC
        hp = h_proj[b0 : b0 + BLOC]  # [BLOC, D]
        # hb[p, dc, b] = hp[b, 128*dc + p]
        hb = np.ascontiguousarray(hp.T.reshape(DCH, 128, BLOC).transpose(1, 0, 2))
        in_maps.append(
            {
                "enc": np.ascontiguousarray(encoder_outputs[:, b0 : b0 + BLOC, :]),
                "wt": wt,
                "hb": hb.astype(np.float32),
                "vw": vw,
            }
        )

    trace = bool(int(os.environ.get("KERNEL_TRACE", "0")))
    res = run_bass_kernel_spmd(
        nc, in_maps, core_ids=list(range(NCORES)), trace=trace
    )
    kernel.last_result = res

    cores = np.stack([res.results[c]["out"] for c in range(NCORES)])  # [NC, BLOC, S]
    full = cores.reshape(B, S).transpose(1, 0).reshape(S, B, 1)
    return np.ascontiguousarray(full, dtype=np.float32)


kernel.last_result = None
